# revision 37
# baseline (speedup 1.0000x reference)
"""Trainium2 Bass kernel for nn_ChannelClustering (vq_codebook).

Problem: image [16, 512, 64, 64] f32, num_clusters=8.
Per batch element:
  1. corr = cosine-similarity matrix of per-channel-standardized image [512, 512]
  2. kmeans (20 Lloyd iters, init = first 8 rows) on corr rows -> labels [512]
  3. medoids: per cluster, channel minimizing mean pairwise L2 distance (on RAW
     image rows); output = those channels' raw data [8, 64, 64].

Key algebraic identity: corr is invariant to per-channel affine scaling, so
  corr[c,d] = (G[c,d] - m_c r_d) / (nu_c nu_d),  nu_c^2 = G[c,c] - m_c r_c
with G = A @ A.T the RAW gram, r = row sums, m = r/S. The SAME gram G gives
the medoid distance matrix d2 = sq_c + sq_d - 2 G (sq = diag G). One
512x512x4096 fp32 gram per batch element drives the whole pipeline; only its
upper-triangular blocks are computed (PE accumulation is bitwise symmetric),
the lower blocks are mirrored by PE transposes. Verified offline
(margin_analysis.py): fp32 arithmetic reproduces the jax reference's argmin
decisions exactly on all 16 batches (kmeans min margin 4.4e-6, medoid min
margin 1.6e-3; corr noise <=1e-6 never flips a decision). f32r matmuls
(~1e-2 rel) and raw ACT Sqrt for the normalization (7e-6 rel) are NOT accurate
enough -- all matmuls are fp32 and 1/nu gets one Newton rsqrt refinement.

The two batch elements' kmeans/medoid stages are FUSED: batch 0 lives on
partitions 0..8 and batch 1 on partitions 32..40 of shared [40,*] tiles, so
every elementwise/reduce op covers both batches in one instruction and the
per-batch matmul pairs land in different PE column groups (tile_position
(0,0)/(0,32)) where the hardware runs them concurrently. Partitions 8..32
carry garbage that no matmul lhsT or output consumer ever reads.

Sharding: pure data parallelism, 2 batch elements per core x 8 cores.
"""
import sys

sys.path.insert(0, "/opt/trn_rl_repo")

import numpy as np

import concourse.bass as bass
import concourse.mybir as mybir
import concourse.tile as tile
from concourse import bacc
from concourse.bass_utils import run_bass_kernel_spmd
from concourse.masks import make_identity

F32 = mybir.dt.float32
I32 = mybir.dt.int32
AF = mybir.ActivationFunctionType
OP = mybir.AluOpType
AX = mybir.AxisListType

C = 512          # channels
S = 4096         # spatial (64*64)
K = 8            # clusters
NB = 2           # batch elements per core
CT = 4           # channel tiles of 128
SC = 32          # spatial chunks of 128
KM_ITERS = 20
BIG = 1.0e9
IBIG = 65536.0   # index-decoy offset (exact in fp32 up to +511)
P1 = 0           # partition base of batch 0 in fused tiles
P2 = 32          # partition base of batch 1 (PE column-group aligned)
W = 40           # fused tile partition extent


def build(nc, km_iters=KM_ITERS, do_medoid=True, do_kmeans=True):
    img = nc.dram_tensor("image", [NB * C, S], F32, kind="ExternalInput")
    out = nc.dram_tensor("out", [NB * K, S], F32, kind="ExternalOutput")

    with tile.TileContext(nc) as tc:
        from contextlib import ExitStack
        ctx = ExitStack()
        const = ctx.enter_context(tc.tile_pool(name="const", bufs=1))
        atp = ctx.enter_context(tc.tile_pool(name="at", bufs=8))
        stagep = ctx.enter_context(tc.tile_pool(name="stage", bufs=6))
        bigp = ctx.enter_context(tc.tile_pool(name="big", bufs=1))
        workp = ctx.enter_context(tc.tile_pool(name="work", bufs=2))
        smallp = ctx.enter_context(tc.tile_pool(name="small", bufs=2))
        kmp = ctx.enter_context(tc.tile_pool(name="km", bufs=2))
        # PSUM budget (8 banks): pg0..pg3 (4, reused by kmeans) + pmix x3 (3) + pk32 (1)
        psum_g = ctx.enter_context(tc.tile_pool(name="ps_g", bufs=1, space="PSUM"))
        psum_mix = ctx.enter_context(tc.tile_pool(name="ps_mix", bufs=3, space="PSUM"))
        psum_k32 = ctx.enter_context(tc.tile_pool(name="ps_k32", bufs=1, space="PSUM"))

        # ---------------- constants ----------------
        ident = const.tile([128, 128], F32, tag="ident")
        make_identity(nc, ident[:])

        # iota over k within each 8-group of the fused [128, 4*W] layout
        iota8_i = const.tile([128, 4 * W], I32, tag="iota8i")
        nc.gpsimd.iota(iota8_i[:].rearrange("p (g q) -> p g q", q=8),
                       pattern=[[0, 4 * W // 8], [1, 8]], base=0, channel_multiplier=0)
        iota8 = const.tile([128, 4 * W], F32, tag="iota8")
        nc.vector.tensor_copy(iota8[:], iota8_i[:])
        iota8b = const.tile([128, 4 * W], F32, tag="iota8b")
        nc.vector.tensor_scalar(iota8b[:], iota8[:], IBIG, None, op0=OP.add)

        iota512_i = const.tile([W, 512], I32, tag="iota512i")
        nc.gpsimd.iota(iota512_i[:], pattern=[[1, 512]], base=0, channel_multiplier=0)
        iota512b = const.tile([W, 512], F32, tag="iota512b")
        nc.vector.tensor_copy(iota512b[:], iota512_i[:])
        nc.vector.tensor_scalar(iota512b[:], iota512b[:], IBIG, None, op0=OP.add)

        ones_col = const.tile([128, 1], F32, tag="ones_col")
        nc.vector.memset(ones_col[:], 1.0)
        ones_row128 = const.tile([1, 128], F32, tag="ones_row128")
        nc.vector.memset(ones_row128[:], 1.0)
        ones_row512 = const.tile([1, 512], F32, tag="ones_row512")
        nc.vector.memset(ones_row512[:], 1.0)
        offs = const.tile([W, 1], F32, tag="offs")
        nc.vector.memset(offs[0:K, :], 0.0)
        nc.vector.memset(offs[P2:P2 + K, :], float(C))

        gat = bigp.tile([W, S], F32, tag="gather")

        g_all = []    # per batch: list of 4 g tiles
        x_all = []    # per batch: list of 4 X tiles
        sq_all = []   # per batch: list of 4 sq column tiles
        for b in range(NB):
            # ==== load + transpose (streamed AT chunks) + triangular gram ====
            pg = [psum_g.tile([128, 512], F32, tag=f"pg{m}", name=f"pg{b}_{m}")
                  for m in range(CT)]
            rpall = smallp.tile([128, 16], F32, tag="rpall")  # r partials (ct, h)
            # software-pipelined: transpose chunk k+1 is emitted before the
            # gram matmuls of chunk k, so the PSUM->SBUF evacuation of chunk k
            # hides under PE transpose work instead of stalling the PE.
            pend = None  # (atk, k) awaiting gram matmuls

            def gram_mms(atk, k):
                for m in range(CT):
                    # upper-triangular blocks only: columns 128m..512
                    nc.tensor.matmul(
                        pg[m][:, 0:512 - 128 * m],
                        lhsT=atk[:, 128 * m:128 * (m + 1)],
                        rhs=atk[:, 128 * m:512],
                        start=(k == 0), stop=(k == SC - 1))

            for h in range(4):
                stages = []
                for ct in range(CT):
                    stg = stagep.tile([128, 1024], F32, tag="stage", name=f"stg{b}_{h}_{ct}")
                    nc.sync.dma_start(
                        out=stg[:],
                        in_=img[b * C + 128 * ct: b * C + 128 * (ct + 1),
                                1024 * h: 1024 * (h + 1)])
                    nc.vector.tensor_reduce(rpall[:, 4 * ct + h: 4 * ct + h + 1],
                                            stg[:], axis=AX.X, op=OP.add)
                    stages.append(stg)
                for kk in range(8):
                    k = 8 * h + kk
                    atk = atp.tile([128, 512], F32, tag="atk", name=f"atk{b}_{k}")
                    pt = psum_mix.tile([128, 512], F32, tag="pmix", name=f"pt{b}_{k}")
                    for ct in range(CT):
                        nc.tensor.transpose(
                            pt[:, 128 * ct:128 * (ct + 1)],
                            stages[ct][:, 128 * kk:128 * (kk + 1)],
                            ident[:])
                    if kk % 2 == 0:
                        nc.scalar.activation(atk[:], pt[:], AF.Copy)
                    else:
                        nc.vector.tensor_copy(atk[:], pt[:])
                    if pend is not None:
                        gram_mms(*pend)
                    pend = (atk, k)
            gram_mms(*pend)
            g_sb = []
            for m in range(CT):
                g_t = bigp.tile([128, 512], F32, tag=f"g{b}_{m}", name=f"g{b}_{m}")
                nc.scalar.activation(g_t[:, 128 * m:512], pg[m][:, 0:512 - 128 * m],
                                     AF.Copy)
                g_sb.append(g_t)
            # mirror lower blocks: g[mp][:, 128m:+128] = transpose(g[m][:, 128mp:+128])
            for m in range(CT):
                for mp in range(m + 1, CT):
                    pmir = psum_mix.tile([128, 128], F32, tag="pmix",
                                         name=f"pmir{b}_{m}_{mp}")
                    nc.tensor.transpose(pmir[:], g_sb[m][:, 128 * mp:128 * (mp + 1)],
                                        ident[:])
                    nc.scalar.activation(g_sb[mp][:, 128 * m:128 * (m + 1)], pmir[:],
                                         AF.Copy)
            g_all.append(g_sb)

            # r columns per channel-tile, r_row [1,512] via PE transposes
            r_cols = []
            for ct in range(CT):
                r_c = smallp.tile([128, 1], F32, tag=f"rcol{b}_{ct}",
                                  name=f"rcol{b}_{ct}")
                nc.vector.tensor_reduce(
                    r_c[:], rpall[:, 4 * ct:4 * ct + 4], axis=AX.X, op=OP.add)
                r_cols.append(r_c)
            r4 = smallp.tile([128, 4], F32, tag="r4")
            for m in range(CT):
                nc.vector.tensor_copy(r4[:, m:m + 1], r_cols[m][:])
            prr = psum_mix.tile([1, 512], F32, tag="pmix", name=f"prr{b}")
            for m in range(CT):
                nc.tensor.transpose(prr[0:1, 128 * m:128 * (m + 1)],
                                    r4[:, m:m + 1], ident[:])
            r_row = smallp.tile([1, 512], F32, tag=f"r_row{b}", name=f"r_row{b}")
            nc.scalar.activation(r_row[:], prr[:], AF.Copy)
            m_row = smallp.tile([1, 512], F32, tag=f"m_row{b}", name=f"m_row{b}")
            nc.scalar.activation(m_row[:], prr[:], AF.Copy, scale=1.0 / S)

            # ==== corr: X = (G - m r^T) * invnu invnu^T ====
            invnu_cols = []
            sq_cols = []
            for m in range(CT):
                scratch = workp.tile([128, 512], F32, tag="scratch")
                nc.gpsimd.affine_select(
                    out=scratch[:], in_=g_sb[m][:], pattern=[[1, 512]],
                    compare_op=OP.is_equal, fill=0.0,
                    base=-128 * m, channel_multiplier=-1)
                sq_c = smallp.tile([128, 1], F32, tag=f"sq{b}_{m}", name=f"sq{b}_{m}")
                nc.vector.tensor_reduce(sq_c[:], scratch[:], axis=AX.X, op=OP.add)
                sq_cols.append(sq_c)
                r2 = smallp.tile([128, 1], F32, tag="r2")
                nc.vector.tensor_tensor(r2[:], r_cols[m][:], r_cols[m][:], op=OP.mult)
                nu2 = smallp.tile([128, 1], F32, tag="nu2")
                nc.vector.scalar_tensor_tensor(
                    out=nu2[:], in0=r2[:], scalar=-1.0 / S, in1=sq_c[:],
                    op0=OP.mult, op1=OP.add)
                # invnu = rsqrt(nu2), Newton-refined (ACT Sqrt alone is ~7e-6)
                y0 = smallp.tile([128, 1], F32, tag="y0")
                nc.scalar.activation(y0[:], nu2[:], AF.Sqrt)
                z0 = smallp.tile([128, 1], F32, tag="z0")
                nc.vector.reciprocal(z0[:], y0[:])
                t_ = smallp.tile([128, 1], F32, tag="t_")
                nc.vector.tensor_tensor(t_[:], z0[:], z0[:], op=OP.mult)
                nc.vector.tensor_tensor(t_[:], t_[:], nu2[:], op=OP.mult)
                nc.vector.tensor_scalar(t_[:], t_[:], -0.5, 1.5, op0=OP.mult, op1=OP.add)
                inv_c = smallp.tile([128, 1], F32, tag=f"invnu{b}_{m}",
                                    name=f"invnu{b}_{m}")
                nc.vector.tensor_tensor(inv_c[:], z0[:], t_[:], op=OP.mult)
                invnu_cols.append(inv_c)
            sq_all.append(sq_cols)

            nu4 = smallp.tile([128, 4], F32, tag="nu4")
            for m in range(CT):
                nc.vector.tensor_copy(nu4[:, m:m + 1], invnu_cols[m][:])
            pnr = psum_mix.tile([1, 512], F32, tag="pmix", name=f"pnr{b}")
            for m in range(CT):
                nc.tensor.transpose(pnr[0:1, 128 * m:128 * (m + 1)],
                                    nu4[:, m:m + 1], ident[:])
            invnu_row = smallp.tile([1, 512], F32, tag="invnu_row")
            nc.scalar.activation(invnu_row[:], pnr[:], AF.Copy)
            pB = psum_mix.tile([128, 512], F32, tag="pmix", name=f"pB{b}")
            nc.tensor.matmul(pB[:], lhsT=ones_row128[:], rhs=invnu_row[:],
                             start=True, stop=True)
            b_sb = workp.tile([128, 512], F32, tag="b_sb")
            nc.scalar.activation(b_sb[:], pB[:], AF.Copy)

            x_sb = []
            for m in range(CT):
                p1t = psum_mix.tile([128, 512], F32, tag="pmix", name=f"p1_{b}_{m}")
                nc.tensor.matmul(p1t[:], lhsT=m_row[0:1, 128 * m:128 * (m + 1)],
                                 rhs=r_row[0:1, :], start=True, stop=True)
                t1 = workp.tile([128, 512], F32, tag="t1")
                nc.vector.scalar_tensor_tensor(
                    out=t1[:], in0=p1t[:], scalar=-1.0, in1=g_sb[m][:],
                    op0=OP.mult, op1=OP.add)
                x_t = bigp.tile([128, 512], F32, tag=f"x{b}_{m}", name=f"x{b}_{m}")
                nc.vector.scalar_tensor_tensor(
                    out=x_t[:], in0=t1[:], scalar=invnu_cols[m][:], in1=b_sb[:],
                    op0=OP.mult, op1=OP.mult)
                x_sb.append(x_t)
            x_all.append(x_sb)

        if not do_kmeans:
            for b in range(NB):
                nc.sync.dma_start(out=out[b * K:(b + 1) * K, 0:512],
                                  in_=x_all[b][0][0:K, :])
            ctx.close()
            return nc

        # D matrices (medoid distances) need only G + sq -- computed here so
        # the scheduler can fill kmeans-chain gaps with this work.
        d_all = []
        for b in range(NB):
            sq4 = smallp.tile([128, 4], F32, tag="sq4")
            for m in range(CT):
                nc.vector.tensor_copy(sq4[:, m:m + 1], sq_all[b][m][:])
            psq = psum_mix.tile([1, 512], F32, tag="pmix", name=f"psq{b}")
            for m in range(CT):
                nc.tensor.transpose(psq[0:1, 128 * m:128 * (m + 1)],
                                    sq4[:, m:m + 1], ident[:])
            sq_row = smallp.tile([1, 512], F32, tag=f"sq_row{b}", name=f"sq_row{b}")
            nc.scalar.activation(sq_row[:], psq[:], AF.Copy)
            d_sb = []
            for m in range(CT):
                pS = psum_mix.tile([128, 512], F32, tag="pmix", name=f"pS{b}_{m}")
                nc.tensor.matmul(pS[:], lhsT=ones_row128[:], rhs=sq_row[:],
                                 start=True, stop=False)
                nc.tensor.matmul(pS[:], lhsT=sq_row[0:1, 128 * m:128 * (m + 1)],
                                 rhs=ones_row512[:], start=False, stop=True)
                d2t = workp.tile([128, 512], F32, tag="d2t")
                nc.vector.scalar_tensor_tensor(
                    out=d2t[:], in0=g_all[b][m][:], scalar=-2.0, in1=pS[:],
                    op0=OP.mult, op1=OP.add)
                d_t = bigp.tile([128, 512], F32, tag=f"d{b}_{m}", name=f"d{b}_{m}")
                nc.scalar.activation(d_t[:], d2t[:], AF.Sqrt)
                d_sb.append(d_t)
            d_all.append(d_sb)

        # ================= fused two-batch kmeans =================
        PB = (P1, P2)
        # init centers: C_b = X_b rows 0..8, placed at partition base PB[b]
        pinit = psum_g.tile([W, 512], F32, tag="pg2", name="pinit")
        for b in range(NB):
            nc.tensor.matmul(pinit[PB[b]:PB[b] + K, :], lhsT=ident[:, 0:K],
                             rhs=x_all[b][0][:], start=True, stop=True)
        c_sb = kmp.tile([W, 512], F32, tag="c_sb")
        nc.scalar.activation(c_sb[:], pinit[:], AF.Copy)

        oh_final = None
        for it in range(km_iters + 1):
            # cnorm2 for both batches in one op
            sc8 = kmp.tile([W, 512], F32, tag="sc8")
            cn2 = kmp.tile([W, 1], F32, tag="cn2")
            nc.scalar.activation(sc8[:], c_sb[:], AF.Square, accum_out=cn2[:])
            # C^T chunks: [W, 128] -> [128, W] per chunk u, packed [128, 4W]
            pct = psum_k32.tile([128, 4 * W], F32, tag="pk32", name=f"pct{it}")
            for u in range(CT):
                nc.tensor.transpose(pct[:, W * u:W * (u + 1)],
                                    c_sb[0:W, 128 * u:128 * (u + 1)],
                                    ident[0:W, 0:W])
            ct_sb = kmp.tile([128, 4 * W], F32, tag="ct_sb")
            nc.scalar.activation(ct_sb[:], pct[:], AF.Copy, scale=-2.0)
            # cn2 as a row [1, W] for the rank-1 bias matmul
            pcr = psum_mix.tile([1, W], F32, tag="pmix", name=f"pcr{it}")
            nc.tensor.transpose(pcr[0:1, 0:W], cn2[0:W, 0:1], ident[0:W, 0:W])
            cn2row = kmp.tile([1, W], F32, tag="cn2row")
            nc.scalar.activation(cn2row[:], pcr[:], AF.Copy)
            # dists DIRECTLY in c-major [128, 4W]: pdt[c,k] = cn2_k +
            # sum_d X[d,c] * (-2C)[k,d], using X's symmetry for the lhsT.
            # Small-N matmuls (N=8) avoid the [W,512] pass + evac + transpose.
            pdt = psum_k32.tile([128, 4 * W], F32, tag="pk32", name=f"pdt{it}")
            for t in range(CT):
                for b in range(NB):
                    col = pdt[:, W * t + PB[b]:W * t + PB[b] + K]
                    for u in range(CT):
                        nc.tensor.matmul(
                            col,
                            lhsT=x_all[b][u][:, 128 * t:128 * (t + 1)],
                            rhs=ct_sb[:, W * u + PB[b]:W * u + PB[b] + K],
                            start=(u == 0), stop=False)
                    # rank-1 cnorm2 bias LAST: the cn2 leg (Square->transpose->
                    # evac) is longer than the ct_sb leg, so the X*C matmuls
                    # must not wait on it
                    nc.tensor.matmul(col, lhsT=ones_row128[:],
                                     rhs=cn2row[0:1, PB[b]:PB[b] + K],
                                     start=False, stop=True)
            # grouped argmin along k (first-index tie-break), all groups at once
            G8 = 4 * W // 8
            dview = pdt[:].rearrange("p (g q) -> p g q", q=8)
            mn = kmp.tile([128, G8], F32, tag="mn")
            nc.vector.tensor_reduce(mn[:], dview, axis=AX.X, op=OP.min)
            mask = kmp.tile([128, 4 * W], F32, tag="mask")
            mn_b = mn[:].rearrange("p (g q) -> p g q", q=1).to_broadcast([128, G8, 8])
            nc.vector.tensor_tensor(mask[:].rearrange("p (g q) -> p g q", q=8),
                                    dview, mn_b, op=OP.is_equal)
            cand = kmp.tile([128, 4 * W], F32, tag="cand")
            nc.vector.scalar_tensor_tensor(
                out=cand[:], in0=mask[:], scalar=-IBIG, in1=iota8b[:],
                op0=OP.mult, op1=OP.add)
            idx = kmp.tile([128, G8], F32, tag="idx")
            nc.vector.tensor_reduce(idx[:], cand[:].rearrange("p (g q) -> p g q", q=8),
                                    axis=AX.X, op=OP.min)
            oh = kmp.tile([128, 4 * W], F32, tag="oh")
            idx_b = idx[:].rearrange("p (g q) -> p g q", q=1).to_broadcast([128, G8, 8])
            nc.vector.tensor_tensor(oh[:].rearrange("p (g q) -> p g q", q=8),
                                    iota8[:].rearrange("p (g q) -> p g q", q=8),
                                    idx_b, op=OP.is_equal)
            if it == km_iters:
                oh_final = oh
                break
            # update: newsum [W,512], cnt [W,1]; batch pairs in column groups
            # newsum^T [128, 4W] in d-major via small-N matmuls (bitwise the
            # same contraction: lhsT = X[c-chunk, d-chunk], rhs = oh slice),
            # then PE-transpose back to [W, 512] for the per-k rec scale.
            pns = psum_g.tile([128, 512], F32, tag="pg0", name=f"pns{it}")
            pnsv = pns[:, 0:4 * W].rearrange("p (t w) -> p t w", w=W)
            for t in range(CT):
                for b in range(NB):
                    colw = pns[:, W * t + PB[b]:W * t + PB[b] + K]
                    for u in range(CT):
                        nc.tensor.matmul(
                            colw,
                            lhsT=x_all[b][u][:, 128 * t:128 * (t + 1)],
                            rhs=oh[:, W * u + PB[b]:W * u + PB[b] + K],
                            start=(u == 0), stop=(u == CT - 1))
            ns_sb = kmp.tile([128, 4 * W], F32, tag="ns_sb")
            nc.scalar.activation(ns_sb[:], pns[:, 0:4 * W], AF.Copy)
            pu = psum_g.tile([W, 512], F32, tag="pg1", name=f"pu{it}")
            for t in range(CT):
                nc.tensor.transpose(pu[0:W, 128 * t:128 * (t + 1)],
                                    ns_sb[:, W * t:W * (t + 1)],
                                    ident[:])
            pc = psum_k32.tile([W, 1], F32, tag="pk32", name=f"pc{it}")
            for t in range(CT):
                for b in range(NB):
                    nc.tensor.matmul(
                        pc[PB[b]:PB[b] + K, :],
                        lhsT=oh[:, W * t + PB[b]:W * t + PB[b] + K],
                        rhs=ones_col[:],
                        start=(t == 0), stop=(t == CT - 1))
            cnt = kmp.tile([W, 1], F32, tag="cnt")
            nc.vector.tensor_scalar(cnt[:], pc[:], 1.0, None, op0=OP.max)
            rec = kmp.tile([W, 1], F32, tag="rec")
            nc.vector.reciprocal(rec[:], cnt[:])
            nc.vector.tensor_scalar(c_sb[:], pu[:], rec[:], None, op0=OP.mult)

        if not do_medoid:
            for b in range(NB):
                nc.sync.dma_start(out=out[b * K:(b + 1) * K, 0:32],
                                  in_=oh_final[0:K, 0:32])
            ctx.close()
            return nc

        # ================= fused medoids =================
        # avgT[k, c] = sum_d oh[d,k] D[d,c] (cnt division skipped: row-uniform)
        pavg = psum_g.tile([W, 512], F32, tag="pg0", name="pavg")
        for u in range(CT):
            for b in range(NB):
                nc.tensor.matmul(
                    pavg[PB[b]:PB[b] + K, :],
                    lhsT=oh_final[:, W * u + PB[b]:W * u + PB[b] + K],
                    rhs=d_all[b][u][:],
                    start=(u == 0), stop=(u == CT - 1))
        # ohT [W, 512]
        poh = psum_g.tile([W, 512], F32, tag="pg1", name="poh")
        for b in range(NB):
            for t in range(CT):
                # regular matmul oh.T @ I (transpose-mode can't target partition 32)
                nc.tensor.matmul(poh[PB[b]:PB[b] + K, 128 * t:128 * (t + 1)],
                                 lhsT=oh_final[:, W * t + PB[b]:W * t + PB[b] + K],
                                 rhs=ident[:], start=True, stop=True)
        ohT = kmp.tile([W, 512], I32, tag="ohT")
        nc.scalar.activation(ohT[:], poh[:], AF.Copy)
        msk = kmp.tile([W, 512], F32, tag="msk")
        nc.vector.memset(msk[:], BIG)
        nc.vector.copy_predicated(msk[:], ohT[:], pavg[:])
        mn8 = kmp.tile([W, 1], F32, tag="mn8")
        nc.vector.tensor_reduce(mn8[:], msk[:], axis=AX.X, op=OP.min)
        mask8 = kmp.tile([W, 512], F32, tag="mask8")
        nc.vector.tensor_tensor(mask8[:], msk[:], mn8[:].to_broadcast([W, 512]),
                                op=OP.is_equal)
        cand8 = kmp.tile([W, 512], F32, tag="cand8")
        nc.vector.scalar_tensor_tensor(
            out=cand8[:], in0=mask8[:], scalar=-IBIG, in1=iota512b[:],
            op0=OP.mult, op1=OP.add)
        idx8 = kmp.tile([W, 1], F32, tag="idx8")
        nc.vector.tensor_reduce(idx8[:], cand8[:], axis=AX.X, op=OP.min)
        nc.vector.tensor_tensor(idx8[:], idx8[:], offs[:], op=OP.add)
        idx_i = kmp.tile([W, 1], I32, tag="idx_i")
        nc.vector.memset(idx_i[:], 0)  # gap partitions gather row 0 harmlessly
        nc.vector.tensor_copy(idx_i[0:K, :], idx8[0:K, :])
        nc.vector.tensor_copy(idx_i[P2:P2 + K, :], idx8[P2:P2 + K, :])

        nc.gpsimd.indirect_dma_start(
            out=gat[:], out_offset=None,
            in_=img[:, :],
            in_offset=bass.IndirectOffsetOnAxis(ap=idx_i[:, 0:1], axis=0))
        for b in range(NB):
            nc.sync.dma_start(out=out[b * K:(b + 1) * K, :],
                              in_=gat[PB[b]:PB[b] + K, :])

        ctx.close()
    return nc


_CACHED = {}


def _get_nc():
    if "nc" not in _CACHED:
        nc = bacc.Bacc("TRN2", target_bir_lowering=False, debug=False)
        build(nc)
        nc.finalize()
        _CACHED["nc"] = nc
    return _CACHED["nc"]


def _run(np_image_16):
    x = np.ascontiguousarray(np_image_16.reshape(16, C, S))
    n_cores = 8
    per = 16 // n_cores
    in_maps = [
        {"image": x[i * per:(i + 1) * per].reshape(per * C, S)}
        for i in range(n_cores)
    ]
    nc = _get_nc()
    return run_bass_kernel_spmd(nc, in_maps, core_ids=list(range(n_cores)))


def kernel(image: np.ndarray, num_clusters) -> np.ndarray:
    assert int(num_clusters) == K
    B, Cc, H, W_ = image.shape
    assert (B, Cc, H * W_) == (16, C, S), image.shape
    res = _run(np.asarray(image, dtype=np.float32))
    per = 2
    outs = [res.results[i]["out"].reshape(per, K, H, W_) for i in range(8)]
    return np.concatenate(outs, axis=0).astype(image.dtype)


def profile_exec_ns(np_inputs):
    """Best-effort HW exec time via traced run; returns ns or None."""
    try:
        x = np.ascontiguousarray(
            np.asarray(np_inputs["image"], np.float32).reshape(16, C, S))
        in_maps = [{"image": x[i * 2:(i + 1) * 2].reshape(2 * C, S)} for i in range(8)]
        nc = _get_nc()
        res = run_bass_kernel_spmd(nc, in_maps, core_ids=list(range(8)), trace=True)
        return res.exec_time_ns
    except Exception as e:
        print(f"profile_exec_ns failed: {type(e).__name__}: {e}")
        return None


if __name__ == "__main__":
    rng = np.random.default_rng(0)
    img = rng.standard_normal((16, C, 64, 64), dtype=np.float32)
    o = kernel(image=img, num_clusters=8)
    print("kernel output shape:", o.shape)
